# revision 7
# baseline (speedup 1.0000x reference)
"""Trainium2 Bass kernel for nn_CrossAttUnit, v4 — further instruction-count cuts.

v4 vs v3:
  - M computed per PAIR of 512-token groups into one [128, 8, 64] PSUM bank;
    softmax stats (neg-max reduce, reciprocal), the rr multiply / +eps, and
    the final normalize all batch over the pair -> roughly half the DVE /
    GPSIMD / stat instructions of v3.
  - one [128, 2, 512] PSUM->SBUF projection copy per group, alternating
    DVE / ACT (v3: two per group).
  - hi+lo f32 reconstruction adds batched per pair ([128, 2, 1024] slices).
Everything else (fp32 projections, HWDGE p-major output, input layout)
matches v3.
"""

import numpy as np

N_TOTAL = 262144
H = 256
D = 64
L = 64
NCORES = 8
N_LOC = N_TOTAL // NCORES  # 32768
GROUP_TOK = 512
SB_GROUPS = 4
SB_TOK = GROUP_TOK * SB_GROUPS  # 2048
SCALE = 0.125
EPS = 1e-6

_CACHE = {}


def _build_program(n_loc):
    import concourse.bacc as bacc
    import concourse.tile as tile
    from concourse import mybir

    f32 = mybir.dt.float32
    bf16 = mybir.dt.bfloat16
    FT = mybir.ActivationFunctionType
    OP = mybir.AluOpType

    nc = bacc.Bacc("TRN2", target_bir_lowering=False)

    yhi_d = nc.dram_tensor("yhi", [2, n_loc, 128], bf16, kind="ExternalInput")
    ylo_d = nc.dram_tensor("ylo", [2, n_loc, 128], bf16, kind="ExternalInput")
    hhi_d = nc.dram_tensor("hhi", [2, n_loc, 128], bf16, kind="ExternalInput")
    hlo_d = nc.dram_tensor("hlo", [2, n_loc, 128], bf16, kind="ExternalInput")
    kx_d = nc.dram_tensor("kx", [128, 2, 128], f32, kind="ExternalInput")
    qx_d = nc.dram_tensor("qx", [128, 2, 128], f32, kind="ExternalInput")
    bd_d = nc.dram_tensor("bdiag", [128, 128], f32, kind="ExternalInput")
    n_sb = n_loc // SB_TOK
    w_d = nc.dram_tensor("w", [128, n_sb * 16 * L], f32, kind="ExternalOutput")

    n_groups = n_loc // GROUP_TOK
    n_pairs = n_groups // 2

    with tile.TileContext(nc) as tc:
        with (
            tc.tile_pool(name="consts", bufs=1) as consts,
            tc.tile_pool(name="ybf", bufs=2) as ybfp,
            tc.tile_pool(name="yf32", bufs=2) as yf32p,
            tc.tile_pool(name="kq", bufs=4) as kqp,
            tc.tile_pool(name="attn", bufs=4) as attnp,
            tc.tile_pool(name="rcs", bufs=2) as rcp,
            tc.tile_pool(name="wout", bufs=2) as woutp,
            tc.tile_pool(name="stats", bufs=6) as statp,
            tc.tile_pool(name="ps_p", bufs=2, space="PSUM") as ps_p,
            tc.tile_pool(name="ps_m", bufs=2, space="PSUM") as ps_m,
            tc.tile_pool(name="ps_c", bufs=2, space="PSUM") as ps_c,
        ):
            kx_sb = consts.tile([128, 2, 128], f32)
            qx_sb = consts.tile([128, 2, 128], f32)
            bd_sb = consts.tile([128, 128], f32)
            eps_sb = consts.tile([128, 1], f32)
            nc.sync.dma_start(out=kx_sb[:], in_=kx_d[:])
            nc.sync.dma_start(out=qx_sb[:], in_=qx_d[:])
            nc.sync.dma_start(out=bd_sb[:], in_=bd_d[:])
            nc.vector.memset(eps_sb[:], EPS)

            ybf_store = {}
            yf32_store = {}
            w_store = {}
            state = {}
            pair_state = {}

            def issue_loads(sb):
                r0 = sb * SB_TOK
                tiles = {}
                for nm, dram_t in (
                    ("yhi", yhi_d),
                    ("ylo", ylo_d),
                    ("hhi", hhi_d),
                    ("hlo", hlo_d),
                ):
                    t = ybfp.tile([128, 2, SB_TOK], bf16, tag=nm, name=nm)
                    for c in range(2):
                        nc.sync.dma_start(
                            out=t[:, c, :],
                            in_=dram_t[c, r0 : r0 + SB_TOK, :],
                            transpose=True,
                        )
                    tiles[nm] = t
                ybf_store[sb] = tiles
                yf32_store[sb] = {
                    "y": yf32p.tile([128, 2, SB_TOK], f32, tag="yf", name="yf"),
                    "h": yf32p.tile([128, 2, SB_TOK], f32, tag="hf", name="hf"),
                }

            def adds_pair(p):
                sb, lp = divmod(p, 2)
                src = ybf_store[sb]
                dst = yf32_store[sb]
                t0, t1 = lp * 2 * GROUP_TOK, (lp + 1) * 2 * GROUP_TOK
                nc.vector.tensor_tensor(
                    dst["y"][:, :, t0:t1],
                    src["yhi"][:, :, t0:t1],
                    src["ylo"][:, :, t0:t1],
                    op=OP.add,
                )
                # DVE, not gpsimd: the Q7 engine is ~4x slower per element and
                # anything queued on it risks stalling consumers (measured).
                # With this, the kernel uses no gpsimd ops at all.
                nc.vector.tensor_tensor(
                    dst["h"][:, :, t0:t1],
                    src["hhi"][:, :, t0:t1],
                    src["hlo"][:, :, t0:t1],
                    op=OP.add,
                )

            def pe_proj(gi):
                sb, g = divmod(gi, SB_GROUPS)
                yf = yf32_store[sb]
                t0, t1 = g * GROUP_TOK, (g + 1) * GROUP_TOK
                ykq_ps = ps_p.tile([128, 2, GROUP_TOK], f32, tag="ykq")
                for slot, (kst, mv) in enumerate(
                    ((kx_sb, yf["y"]), (qx_sb, yf["h"]))
                ):
                    for c in range(2):
                        nc.tensor.matmul(
                            ykq_ps[:, slot, :],
                            kst[:, c, :],
                            mv[:, c, t0:t1],
                            start=(c == 0),
                            stop=(c == 1),
                        )
                state[gi] = {"ykq_ps": ykq_ps}

            def copies(gi):
                # DVE always: ACT is the busier engine (8 exps per pair).
                st = state[gi]
                ykq_sb = kqp.tile([128, 2, GROUP_TOK], f32, tag="ykq_sb")
                nc.vector.tensor_copy(ykq_sb[:], st["ykq_ps"][:])
                st["ykq_sb"] = ykq_sb

            def pe_seg(gi):
                st = state.pop(gi)
                ykq_sb = st["ykq_sb"]
                j, half = divmod(gi, 2)
                if half == 0:
                    M_ps = ps_m.tile([128, 8, L], f32, tag="M", name="M")
                    pair_state[j] = {"M_ps": M_ps}
                else:
                    M_ps = pair_state[j]["M_ps"]
                for s in range(8):
                    sl = slice(s * L, (s + 1) * L)
                    nc.tensor.matmul(
                        M_ps[(s % 2) * 64 : (s % 2) * 64 + 64, half * 4 + s // 2, :],
                        ykq_sb[:, 0, sl],
                        ykq_sb[:, 1, sl],
                        start=True,
                        stop=True,
                    )

            def softmax_front(j):
                ps = pair_state[j]
                M_ps = ps["M_ps"]
                Apair = attnp.tile([128, 8, L], f32, tag="A", name="A")
                ps["A"] = Apair
                nmax = statp.tile([128, 8], f32, tag="nmax")
                rowsum = statp.tile([128, 8], f32, tag="rowsum")
                rr = statp.tile([128, 8], f32, tag="rr")
                nc.vector.tensor_reduce(
                    nmax[:], M_ps[:], axis=mybir.AxisListType.X, op=OP.max, negate=True
                )
                for b in range(8):
                    nc.scalar.activation(
                        Apair[:, b, :],
                        M_ps[:, b, :],
                        FT.Exp,
                        bias=nmax[:, b : b + 1],
                        scale=1.0,
                        accum_out=rowsum[:, b : b + 1],
                    )
                nc.vector.reciprocal(rr[:], rowsum[:])
                # T' = E*rr on DVE, +EPS on ACT: keeps the slow GPSIMD queue
                # (h-adds) off the T' -> colsum critical path entirely.
                rrb = rr[:].unsqueeze(2).broadcast_to((128, 8, L))
                nc.vector.tensor_tensor(Apair[:], Apair[:], rrb, op=OP.mult)
                nc.scalar.activation(Apair[:], Apair[:], FT.Identity, bias=eps_sb[:])

            def pe_colsum(j):
                ps = pair_state[j]
                CS_ps = ps_c.tile([128, 8, L], f32, tag="CS")
                nc.tensor.matmul(CS_ps[:], bd_sb[:], ps["A"][:], start=True, stop=True)
                ps["CS_ps"] = CS_ps

            def norm_back(j):
                ps = pair_state.pop(j)
                sb, jj = divmod(j, 2)
                rc_sb = rcp.tile([128, 8, L], f32, tag="rc")
                nc.vector.reciprocal(rc_sb[:], ps["CS_ps"][:])
                W_super = w_store[sb]
                nc.vector.tensor_tensor(
                    W_super[:, jj * 8 : jj * 8 + 8, :],
                    ps["A"][:],
                    rc_sb[:],
                    op=OP.mult,
                )

            def store_w(sb):
                nc.sync.dma_start(
                    out=w_d[:, sb * 16 * L : (sb + 1) * 16 * L].rearrange(
                        "p (t m) -> p t m", t=16
                    ),
                    in_=w_store.pop(sb)[:],
                )

            # One extra pipeline stage between the projection copy (DVE) and
            # the segment matmuls so the PE queue never stalls on the copy
            # semaphore: seg consumes 2-iteration-old projections.
            issue_loads(0)
            adds_pair(0)
            for gi in range(n_groups + 5):
                sb, g = divmod(gi, SB_GROUPS)
                if gi < n_groups:
                    if g == 0:
                        w_store[sb] = woutp.tile([128, 16, L], f32, tag="W", name="W")
                        if sb + 1 < n_sb:
                            issue_loads(sb + 1)
                    pe_proj(gi)
                if gi >= 2 and gi - 2 < n_groups:
                    pe_seg(gi - 2)
                if gi >= 5 and (gi - 5) % 2 == 0 and (gi - 5) // 2 < n_pairs:
                    pe_colsum((gi - 5) // 2)
                if gi >= 4 and (gi - 4) % 2 == 0 and (gi - 4) // 2 < n_pairs:
                    softmax_front((gi - 4) // 2)
                if gi >= 6 and (gi - 6) % 2 == 0 and (gi - 6) // 2 < n_pairs:
                    j = (gi - 6) // 2
                    norm_back(j)
                    if j % 2 == 1:
                        store_w(j // 2)
                if gi < n_groups:
                    copies(gi)
                if (gi + 2) % 2 == 0 and 2 <= gi + 2 < n_groups:
                    adds_pair((gi + 2) // 2)

    nc.compile()
    return nc


def _split_hi_lo(x):
    import ml_dtypes

    bf = ml_dtypes.bfloat16
    hi = x.astype(bf)
    lo = (x - hi.astype(np.float32)).astype(bf)
    return hi, lo


def _chunk_major(x):
    n = x.shape[0]
    return np.ascontiguousarray(x.reshape(n, 2, 128).transpose(1, 0, 2))


def _pad_proj_f32(m, scale=1.0):
    m = np.asarray(m, dtype=np.float32) * scale
    o = np.zeros((128, 2, 128), dtype=np.float32)
    o[:, 0, 0:64] = m[0:128, :]
    o[:, 1, 0:64] = m[128:256, :]
    return o


def _consts():
    bdiag = np.zeros((128, 128), dtype=np.float32)
    bdiag[:64, :64] = 1.0
    bdiag[64:, 64:] = 1.0
    return bdiag


def _get_program(n_loc):
    if n_loc not in _CACHE:
        _CACHE[n_loc] = _build_program(n_loc)
    return _CACHE[n_loc]


def _prepare(yhat_embedding, y_embedding, k, q):
    nc = _get_program(N_LOC)
    bdiag = _consts()
    y = np.asarray(y_embedding, dtype=np.float32)
    yh = np.asarray(yhat_embedding, dtype=np.float32)
    yhi, ylo = _split_hi_lo(y)
    hhi, hlo = _split_hi_lo(yh)
    kx = _pad_proj_f32(k, scale=SCALE)
    qx = _pad_proj_f32(q)
    in_maps = []
    for i in range(NCORES):
        sl = slice(i * N_LOC, (i + 1) * N_LOC)
        in_maps.append(
            {
                "yhi": _chunk_major(yhi[sl]),
                "ylo": _chunk_major(ylo[sl]),
                "hhi": _chunk_major(hhi[sl]),
                "hlo": _chunk_major(hlo[sl]),
                "kx": kx,
                "qx": qx,
                "bdiag": bdiag,
            }
        )
    return nc, in_maps


def _unpermute(w):
    n_sb = N_LOC // SB_TOK
    return np.ascontiguousarray(
        w.reshape(128, n_sb, 16, L).transpose(1, 2, 0, 3)
    ).reshape(N_LOC, L)


def _run(yhat_embedding, y_embedding, k, q, trace=False):
    from concourse.bass_utils import run_bass_kernel_spmd

    nc, in_maps = _prepare(yhat_embedding, y_embedding, k, q)
    res = run_bass_kernel_spmd(nc, in_maps, core_ids=list(range(NCORES)), trace=trace)
    w = np.concatenate([_unpermute(r["w"]) for r in res.results], axis=0)
    out = w.reshape(N_TOTAL // L, L, L)
    return out, res


def kernel(**inputs):
    yhat_embedding = inputs["yhat_embedding"]
    y_embedding = inputs["y_embedding"]
    k = inputs["k"]
    q = inputs["q"]
    seg_len = int(inputs.get("seg_len", L))
    assert seg_len == L, f"kernel hardcodes seg_len={L}, got {seg_len}"
    out, _ = _run(yhat_embedding, y_embedding, k, q, trace=False)
    return out
